# revision 4
# baseline (speedup 1.0000x reference)
"""Trainium2 Bass kernel: 14-qubit data-reuploading quantum circuit actor.

Circuit per layer l (NL=5):
  for w in 0..13:  RY(in_scale[l,w]*x[:,w]) ; RZ(in_scale[l,w+14]*x[:,w]) on wire w
  for w in 0..13:  RZ(weights[l,w]) on wire w          (merged into input RZ)
  for w in 0..13:  RY(weights[l,w+14]) on wire w
  CNOT ring (i -> i+1 mod 14)
Then <Z_w> for w in 0..5, * action_scale + action_bias.

Strategy (per core; 8 cores data-parallel over batch 2048 -> 256):
  - state = two fp32 SBUF planes [128 batch-partitions, 16384 amplitudes]
  - host precomputes scale/tan tables (no on-chip transcendentals)
  - rotation [[c,-s],[s,c]] = c * [[1,-t],[t,1]] (t = tan(half-angle)):
    full-plane scale by c runs on ScalarE (in-place, merged RY*RZ per wire),
    the two shear rows are single-scalar fused DVE axpy ops (dst = t*y + x)
  - RZ_in * RZ_weight merged (both diagonal on same wire; commute across wires)
  - ring CNOT(t-1,t) folded into RY_w(t) write APs (adjacent bits -> rank-3 APs)
  - CNOT(13,0) folded into next layer's RY_in(0) read APs; explicit on last layer
  - layer-0 gates restricted to the growing support of |0..0>
  - measurement reduced on-chip to 64 block-sums; +/- signs applied on host
"""

import os
import numpy as np

NQ = 14
NL = 5
OBS = 14
NA = 6
B = 2048
NCORES = 8
BPC = B // NCORES          # 256 batch per core
PT = 128                   # partitions (batch rows) per tile
NTILES = BPC // PT         # 2
NS = 1 << NQ               # 16384
HALF = NS // 2
NCPG = 11                  # columns per (layer, wire)
NCOLS = NL * NQ * NCPG     # 770
CHUNK = 4096               # elems per TT-accumulate chunk (stock emitter)

# ---------------------------------------------------------------- host tables


def angle_table(x, input_scaling, weights):
    """(n, NCOLS) f32; per (l,w) 11 cols:
    0: c_ry*C_rz (merged plane scale)   1: t_ry   2: -t_ry
    3: C_rz                             4: t_rz   5: -t_rz
    6: c_wy                             7: t_wy   8: -t_wy
    9: s_ry (layer-0 support)          10: c_ry (layer-0 support)"""
    x = np.asarray(x, np.float64)
    isc = np.asarray(input_scaling, np.float64)
    wt = np.asarray(weights, np.float64)
    n = x.shape[0]
    tab = np.zeros((n, NL, NQ, NCPG), np.float64)
    for l in range(NL):
        for w in range(NQ):
            a_ry = isc[l, w] * x[:, w] / 2.0
            a_rz = (isc[l, w + OBS] * x[:, w] + wt[l, w]) / 2.0
            a_wy = np.full(n, wt[l, w + NQ] / 2.0)
            tab[:, l, w, 0] = np.cos(a_ry) * np.cos(a_rz)
            tab[:, l, w, 1] = np.tan(a_ry)
            tab[:, l, w, 2] = -np.tan(a_ry)
            tab[:, l, w, 3] = np.cos(a_rz)
            tab[:, l, w, 4] = np.tan(a_rz)
            tab[:, l, w, 5] = -np.tan(a_rz)
            tab[:, l, w, 6] = np.cos(a_wy)
            tab[:, l, w, 7] = np.tan(a_wy)
            tab[:, l, w, 8] = -np.tan(a_wy)
            tab[:, l, w, 9] = np.sin(a_ry)
            tab[:, l, w, 10] = np.cos(a_ry)
    return tab.reshape(n, NCOLS).astype(np.float32)


def col(l, w, k):
    return (l * NQ + w) * NCPG + k


def postprocess(s64, action_scale, action_bias):
    """s64: (n, 64) block sums (blocks = top-6 bits). -> (n, NA) f32."""
    blk = np.arange(64)
    out = np.zeros((s64.shape[0], NA), np.float32)
    for w in range(NA):
        sign = 1.0 - 2.0 * ((blk >> (5 - w)) & 1)
        out[:, w] = s64 @ sign.astype(np.float32)
    return out * np.asarray(action_scale, np.float32) + np.asarray(
        action_bias, np.float32
    )


# ------------------------------------------------------------- gate schedule
# region = (buf, offset, dims) ; dims = tuple of (step, count), innermost last
# buf in {"r", "i", "T"}
# ops:
#   ("axpy", dst, y, cty, x): dst = col[cty]*y + x   (dst may alias x or y)
#   ("ts",   dst, src, c):    dst = col[c]*src       (DVE; dst==src ok)
#   ("tsr",  reg, c):         reg *= col[c]          (ScalarE, in place)
#   ("cp",   dst, src):       dst = src              (ScalarE; disjoint)
#   ("mul"/"add", dst, a, b): elementwise TT         (dst==a ok)
#   ("red",  dst, src): dst[.., k] = sum over innermost dim of src


def _rsize(reg):
    n = 1
    for _, c in reg[2]:
        n *= c
    return n


def _h(buf, w, bit):
    """half-region of wire w (bit==0/1), full state."""
    s = 1 << (13 - w)
    p = 1 << (14 - w)
    return (buf, bit * s, ((p, 1 << w), (1, s)))


def _full(buf):
    return (buf, 0, ((1, NS),))


def _conform(reg, like):
    """reshape a contiguous region to the dims-shape of `like` (same size)."""
    buf, off, dims = reg
    assert len(dims) == 1 and dims[0][0] == 1
    shape = tuple(c for _, c in like[2])
    ndims = []
    inner = 1
    for c in reversed(shape):
        ndims.append((inner, c))
        inner *= c
    return (buf, off, tuple(reversed(ndims)))


class Sched:
    def __init__(self):
        self.ops = []

    # shear pair: X' = X + (-t)*Y -> T (copy back), Y' = t*X + Y in place
    def shear2(self, X, Y, ct, cnt):
        n = _rsize(X)
        T = _conform(("T", 0, ((1, n),)), X)
        self.ops.append(("axpy", T, Y, cnt, X))
        self.ops.append(("axpy", Y, X, ct, Y))
        self.ops.append(("cp", X, T))

    def ry(self, w, ct, cnt):
        for P in ("r", "i"):
            self.shear2(_h(P, w, 0), _h(P, w, 1), ct, cnt)

    def rz(self, w, ct, cnt):
        # pairs (r,i): bit0 half rotates by +, bit1 half by - (conjugate)
        self.shear2(_h("r", w, 0), _h("i", w, 0), cnt, ct)
        self.shear2(_h("r", w, 1), _h("i", w, 1), ct, cnt)

    def scale_state(self, c):
        self.ops.append(("tsr", _full("r"), c))
        self.ops.append(("tsr", _full("i"), c))

    # layer-0 support-restricted variants ------------------------------------
    def ry_support(self, w, cc, cs):
        s = 1 << (13 - w)
        p = 1 << (14 - w)
        for P in ("r", "i"):
            i0 = (P, 0, ((p, 1 << w),))
            i1 = (P, s, ((p, 1 << w),))
            self.ops.append(("ts", i1, i0, cs))
            self.ops.append(("ts", i0, i0, cc))

    def rz_support(self, w, cC, ct, cnt):
        s = 1 << (13 - w)
        p = 1 << (14 - w)
        for P in ("r", "i"):
            sup = (P, 0, ((s, 1 << (w + 1)),))
            self.ops.append(("tsr", sup, cC))
        re_e = ("r", 0, ((p, 1 << w),))
        im_e = ("i", 0, ((p, 1 << w),))
        re_o = ("r", s, ((p, 1 << w),))
        im_o = ("i", s, ((p, 1 << w),))
        self.shear2(re_e, im_e, cnt, ct)
        self.shear2(re_o, im_o, ct, cnt)

    # RY_w(t) with CNOT(t-1, t) folded into writes ---------------------------
    def ry_fold_cnot(self, t, ct, cnt):
        st = 1 << (13 - t)
        pt = 1 << (14 - t)
        P2 = pt * 2
        nb = 1 << (t - 1)

        def A(buf, a, b):
            return (buf, a * pt + b * st, ((P2, nb), (1, st)))

        n = nb * st  # 4096
        for P in ("r", "i"):
            T0 = _conform(("T", 0, ((1, n),)), A(P, 0, 0))
            T1 = _conform(("T", n, ((1, n),)), A(P, 0, 0))
            self.ops.append(("axpy", T0, A(P, 0, 1), cnt, A(P, 0, 0)))
            self.ops.append(("axpy", T1, A(P, 1, 1), cnt, A(P, 1, 0)))
            self.ops.append(("axpy", A(P, 0, 1), A(P, 0, 0), ct, A(P, 0, 1)))
            self.ops.append(("axpy", A(P, 1, 0), A(P, 1, 0), ct, A(P, 1, 1)))
            self.ops.append(("cp", A(P, 0, 0), T0))
            self.ops.append(("cp", A(P, 1, 1), T1))

    # RY_in(0) with previous layer's CNOT(13, 0) folded into reads -----------
    def ry0_fold_cnot(self, ct, cnt):
        def A(buf, a, b):  # a = bit0 (MSB), b = bit13 (LSB)
            return (buf, a * HALF + b, ((2, HALF // 2),))

        for P in ("r", "i"):
            Te = ("T", 0, ((2, HALF // 2),))
            To = ("T", 1, ((2, HALF // 2),))
            self.ops.append(("axpy", Te, A(P, 1, 0), cnt, A(P, 0, 0)))
            self.ops.append(("axpy", To, A(P, 0, 1), cnt, A(P, 1, 1)))
            self.ops.append(("axpy", A(P, 1, 0), A(P, 0, 0), ct, A(P, 1, 0)))
            self.ops.append(("axpy", A(P, 1, 1), A(P, 1, 1), ct, A(P, 0, 1)))
            self.ops.append(("cp", (P, 0, ((1, HALF),)), ("T", 0, ((1, HALF),))))

    def cnot_13_0_explicit(self):
        for P in ("r", "i"):
            A01 = (P, 1, ((2, HALF // 2),))
            A11 = (P, HALF + 1, ((2, HALF // 2),))
            Tq = ("T", 0, ((1, HALF // 2),))
            self.ops.append(("cp", Tq, A01))
            self.ops.append(("cp", A01, A11))
            self.ops.append(("cp", A11, Tq))

    def measurement(self):
        self.ops.append(("mul", _full("r"), _full("r"), _full("r")))
        self.ops.append(("mul", _full("i"), _full("i"), _full("i")))
        self.ops.append(("add", _full("r"), _full("r"), _full("i")))
        self.ops.append(("red", ("S", 0, ((1, 64),)), ("r", 0, ((256, 64), (1, 256)))))


def build_schedule():
    S = Sched()
    for l in range(NL):
        for w in range(NQ):
            kc = lambda k: col(l, w, k)  # noqa: E731
            if l == 0:
                S.ry_support(w, kc(10), kc(9))
                S.rz_support(w, kc(3), kc(4), kc(5))
            elif w == 0:
                S.scale_state(kc(0))
                S.ry0_fold_cnot(kc(1), kc(2))
                S.rz(w, kc(4), kc(5))
            else:
                S.scale_state(kc(0))
                S.ry(w, kc(1), kc(2))
                S.rz(w, kc(4), kc(5))
        # weight RY block with folded ring CNOTs
        S.scale_state(col(l, 0, 6))
        S.ry(0, col(l, 0, 7), col(l, 0, 8))
        for t in range(1, NQ):
            S.scale_state(col(l, t, 6))
            S.ry_fold_cnot(t, col(l, t, 7), col(l, t, 8))
    S.cnot_13_0_explicit()
    S.measurement()
    return S.ops


# ------------------------------------------------------------ numpy executor


def _indices(reg):
    _, off, dims = reg
    idx = np.array([0], np.int64)
    for st, ct in dims:
        idx = (idx[:, None] + (np.arange(ct, dtype=np.int64) * st)[None, :]).ravel()
    return off + idx


def simulate_numpy(tab):
    """tab: (n, NCOLS) f32 angle table -> (n, 64) block sums, fp32 ops."""
    n = tab.shape[0]
    bufs = {
        "r": np.zeros((n, NS), np.float32),
        "i": np.zeros((n, NS), np.float32),
        "T": np.zeros((n, HALF), np.float32),
        "S": np.zeros((n, 64), np.float32),
    }
    bufs["r"][:, 0] = 1.0
    A = tab
    for op in build_schedule():
        kind = op[0]
        if kind == "axpy":
            _, dst, y, cty, x = op
            v = (
                A[:, cty : cty + 1] * bufs[y[0]][:, _indices(y)]
                + bufs[x[0]][:, _indices(x)]
            ).astype(np.float32)
            bufs[dst[0]][:, _indices(dst)] = v
        elif kind == "ts":
            _, dst, src, c = op
            bufs[dst[0]][:, _indices(dst)] = (
                A[:, c : c + 1] * bufs[src[0]][:, _indices(src)]
            ).astype(np.float32)
        elif kind == "tsr":
            _, reg, c = op
            ix = _indices(reg)
            bufs[reg[0]][:, ix] = (A[:, c : c + 1] * bufs[reg[0]][:, ix]).astype(
                np.float32
            )
        elif kind == "cp":
            _, dst, src = op
            bufs[dst[0]][:, _indices(dst)] = bufs[src[0]][:, _indices(src)]
        elif kind == "mul":
            _, dst, a, b = op
            bufs[dst[0]][:, _indices(dst)] = (
                bufs[a[0]][:, _indices(a)] * bufs[b[0]][:, _indices(b)]
            ).astype(np.float32)
        elif kind == "add":
            _, dst, a, b = op
            bufs[dst[0]][:, _indices(dst)] = (
                bufs[a[0]][:, _indices(a)] + bufs[b[0]][:, _indices(b)]
            ).astype(np.float32)
        elif kind == "red":
            _, dst, src = op
            v = bufs[src[0]][:, _indices(src)].reshape(n, 64, 256).sum(axis=2)
            bufs[dst[0]][:, _indices(dst)] = v.astype(np.float32)
        else:
            raise ValueError(kind)
    return bufs["S"].copy()


# ------------------------------------------------------------------ bass side

_EMIT_MODE = os.environ.get("QK_EMIT", "custom")  # "custom" | "stock"
_AXPY = None


def _get_axpy():
    """Register the fused out = s0*in0 + in1 custom DVE op (idempotent)."""
    global _AXPY
    if _AXPY is not None:
        return _AXPY
    from concourse.dve_spec import Spec, Src0, Src1, C0, lower
    from concourse.dve_uop import DveOpSpec
    from concourse import dve_ops
    from concourse.dve_ops import DveOp, OPS

    for op in OPS:
        if op.name == "AXPY_ANT":
            _AXPY = op
            return op
    spec = Spec(
        body=Src0 * C0 + Src1,
        reference=lambda in0, in1, s0, s1, imm2: (
            np.asarray(in0, np.float32) * np.asarray(s0, np.float32)
            + np.asarray(in1, np.float32)
        ).astype(np.float32),
    )
    row = dve_ops._CUSTOM_DVE_ROW_BASE + len(OPS)
    shas = {}
    for ver in ("v3", "v4"):
        shas[ver] = DveOpSpec(
            name="AXPY_ANT", opcode=row, uops=lower(spec, ver=ver), rd1_en=True
        ).sha(ver)
    op = DveOp("AXPY_ANT", spec, subdim=False, uops_sha=shas)
    OPS.append(op)
    dve_ops._SUB_OPCODE_FOR_NAME[op.name] = row
    dve_ops.CUSTOM_DVE_SPECS[op.name] = op.spec
    _AXPY = op
    return op


def _ap(bass_mod, tiles, reg):
    """region -> bass AP on the tile's underlying tensor."""
    tile_ap = tiles[reg[0]]
    t = tile_ap.tensor
    part = list(tile_ap.ap)[0]
    dims = [[part[0], part[1]]] + [[s, c] for s, c in reg[2]]
    base = tile_ap.offset
    return bass_mod.AP(t, base + reg[1], dims)


def _chunks(reg):
    """split region into <=CHUNK-element pieces along blocks or inner run."""
    buf, off, dims = reg
    if len(dims) == 1:
        st, ct = dims[0]
        if ct <= CHUNK:
            return [reg]
        assert ct % CHUNK == 0
        return [
            (buf, off + k * CHUNK * st, ((st, CHUNK),)) for k in range(ct // CHUNK)
        ]
    (bs, nb), (st, run) = dims
    assert st == 1
    if nb * run <= CHUNK:
        return [reg]
    if run >= CHUNK:
        assert run % CHUNK == 0
        out = []
        for b in range(nb):
            for k in range(run // CHUNK):
                out.append((buf, off + b * bs + k * CHUNK, ((1, CHUNK),)))
        return out
    bpc = max(1, CHUNK // run)
    out = []
    for b0 in range(0, nb, bpc):
        nbb = min(bpc, nb - b0)
        out.append((buf, off + b0 * bs, ((bs, nbb), (1, run))))
    return out


def build_bass():
    import concourse.bass as bass
    import concourse.mybir as mybir
    import concourse.tile as tile
    from concourse import bacc
    from contextlib import ExitStack

    f32 = mybir.dt.float32
    copy_fn = mybir.ActivationFunctionType.Copy
    nc = bacc.Bacc("TRN2", target_bir_lowering=False, debug=False)
    ang_d = nc.dram_tensor("ang", [BPC, NCOLS], f32, kind="ExternalInput").ap()
    out_d = nc.dram_tensor("out", [BPC, 64], f32, kind="ExternalOutput").ap()

    sched = build_schedule()
    use_custom = _EMIT_MODE == "custom"
    axpy_op = _get_axpy() if use_custom else None

    with tile.TileContext(nc) as tc, ExitStack() as ctx:
        state_p = ctx.enter_context(tc.tile_pool(name="state", bufs=1))
        tmp_p = ctx.enter_context(tc.tile_pool(name="tmp", bufs=1))
        io_p = ctx.enter_context(tc.tile_pool(name="io", bufs=2))

        re_t = state_p.tile([PT, NS], f32, tag="re")
        im_t = state_p.tile([PT, NS], f32, tag="im")
        T_t = tmp_p.tile([PT, HALF], f32, tag="T")
        T2_t = tmp_p.tile([PT, CHUNK], f32, tag="T2")
        for t in range(NTILES):
            ang_t = io_p.tile([PT, NCOLS], f32, tag="ang")
            s64_t = io_p.tile([PT, 64], f32, tag="s64")
            nc.sync.dma_start(ang_t[:], ang_d[t * PT : (t + 1) * PT, :])

            tiles = {"r": re_t[:], "i": im_t[:], "T": T_t[:], "S": s64_t[:]}
            nc.vector.memset(re_t[:, 0:1], 1.0)
            nc.vector.memset(im_t[:, 0:1], 0.0)

            def scal(c):
                return ang_t[:, c : c + 1]

            for op in sched:
                kind = op[0]
                if kind == "axpy":
                    _, dst, y, cty, x = op
                    if use_custom:
                        nc.vector._custom_dve(
                            axpy_op,
                            out=_ap(bass, tiles, dst),
                            in0=_ap(bass, tiles, y),
                            in1=_ap(bass, tiles, x),
                            s0=scal(cty),
                        )
                    else:
                        dcs = _chunks(dst)
                        ycs = _chunks(y)
                        xcs = _chunks(x)
                        for dch, ych, xch in zip(dcs, ycs, xcs, strict=True):
                            n = _rsize(dch)
                            t2 = _conform(("T2", 0, ((1, n),)), dch)
                            t2ap = _ap(bass, {"T2": T2_t[:]}, t2)
                            nc.vector.tensor_scalar_mul(
                                t2ap, _ap(bass, tiles, ych), scal(cty)
                            )
                            nc.vector.tensor_add(
                                _ap(bass, tiles, dch), _ap(bass, tiles, xch), t2ap
                            )
                elif kind == "ts":
                    _, dst, src, c = op
                    nc.vector.tensor_scalar_mul(
                        _ap(bass, tiles, dst), _ap(bass, tiles, src), scal(c)
                    )
                elif kind == "tsr":
                    _, reg, c = op
                    rap = _ap(bass, tiles, reg)
                    nc.scalar.activation(rap, rap, copy_fn, scale=scal(c))
                elif kind == "cp":
                    _, dst, src = op
                    nc.scalar.mul(_ap(bass, tiles, dst), _ap(bass, tiles, src), 1.0)
                elif kind == "mul":
                    _, dst, a, b = op
                    nc.vector.tensor_mul(
                        _ap(bass, tiles, dst), _ap(bass, tiles, a), _ap(bass, tiles, b)
                    )
                elif kind == "add":
                    _, dst, a, b = op
                    nc.vector.tensor_add(
                        _ap(bass, tiles, dst), _ap(bass, tiles, a), _ap(bass, tiles, b)
                    )
                elif kind == "red":
                    _, dst, src = op
                    nc.vector.tensor_reduce(
                        _ap(bass, tiles, dst),
                        _ap(bass, tiles, src),
                        axis=mybir.AxisListType.X,
                        op=mybir.AluOpType.add,
                    )
            nc.sync.dma_start(out_d[t * PT : (t + 1) * PT, :], s64_t[:])
    nc.compile()
    return nc


_NC_CACHE = None


def run_cores(ang_full, trace=False, **kw):
    """ang_full: (B, NCOLS). Returns (B, 64) block sums + BassKernelResults."""
    global _NC_CACHE
    from concourse.bass_utils import run_bass_kernel_spmd

    if _NC_CACHE is None:
        _NC_CACHE = build_bass()
    nc = _NC_CACHE
    in_maps = [
        {"ang": np.ascontiguousarray(ang_full[c * BPC : (c + 1) * BPC])}
        for c in range(NCORES)
    ]
    res = run_bass_kernel_spmd(nc, in_maps, core_ids=list(range(NCORES)),
                               trace=trace, **kw)
    s64 = np.concatenate([r["out"] for r in res.results], axis=0)
    return s64, res


def kernel(x, input_scaling, weights, action_scale, action_bias):
    tab = angle_table(x, input_scaling, weights)
    s64, _ = run_cores(tab)
    return postprocess(s64, action_scale, action_bias)


# revision 29
# speedup vs baseline: 1.4220x; 1.4220x over previous
"""Trainium2 Bass kernel: 14-qubit data-reuploading quantum circuit actor.

Circuit per layer l (NL=5):
  for w in 0..13:  RY(in_scale[l,w]*x[:,w]) ; RZ(in_scale[l,w+14]*x[:,w]) on wire w
  for w in 0..13:  RZ(weights[l,w]) on wire w          (merged into input RZ)
  for w in 0..13:  RY(weights[l,w+14]) on wire w
  CNOT ring (i -> i+1 mod 14)
Then <Z_w> for w in 0..5, * action_scale + action_bias.

Strategy (per core; 8 cores data-parallel over batch 2048 -> 256):
  - state = two fp32 SBUF planes [128 batch-partitions, 16384 amplitudes]
  - host precomputes scale/tan tables (no on-chip transcendentals)
  - rotation [[c,-s],[s,c]] = c * [[1,-t],[t,1]] (t = tan(half-angle)):
    full-plane scale by c runs on ScalarE (in-place, merged RY*RZ per wire),
    the two shear rows are single-scalar fused DVE axpy ops (dst = t*y + x)
  - RZ_in * RZ_weight merged (both diagonal on same wire; commute across wires)
  - ring CNOT(t-1,t) folded into RY_w(t) write APs (adjacent bits -> rank-3 APs)
  - CNOT(13,0) folded into next layer's RY_in(0) read APs; explicit on last layer
  - layer-0 gates restricted to the growing support of |0..0>
  - measurement reduced on-chip to 64 block-sums; +/- signs applied on host
"""

import os
import numpy as np

NQ = 14
NL = 5
OBS = 14
NA = 6
B = 2048
NCORES = 8
BPC = B // NCORES          # 256 batch per core
PT = 128                   # partitions (batch rows) per tile
NTILES = BPC // PT         # 2
NS = 1 << NQ               # 16384
HALF = NS // 2
NCPG = 11                  # columns per (layer, wire)
NCOLS = NL * NQ * NCPG     # 770
CHUNK = 4096               # elems per TT-accumulate chunk (stock emitter)

# ---------------------------------------------------------------- host tables


def angle_table(x, input_scaling, weights):
    """(n, NCOLS) f32; per (l,w) 11 cols:
    1: t_ry   2: -t_ry   4: t_rz   5: -t_rz   7: t_wy   8: -t_wy
    col(l,0,9): whole-layer deferred scale prod_w c_ry*C_rz*c_wy"""
    x = np.asarray(x, np.float64)
    isc = np.asarray(input_scaling, np.float64)
    wt = np.asarray(weights, np.float64)
    n = x.shape[0]
    tab = np.zeros((n, NL, NQ, NCPG), np.float64)
    for l in range(NL):
        lscale = np.ones(n, np.float64)
        for w in range(NQ):
            a_ry = isc[l, w] * x[:, w] / 2.0
            a_rz = (isc[l, w + OBS] * x[:, w] + wt[l, w]) / 2.0
            a_wy = np.full(n, wt[l, w + NQ] / 2.0)
            lscale = lscale * np.cos(a_ry) * np.cos(a_rz) * np.cos(a_wy)
            tab[:, l, w, 1] = np.tan(a_ry)
            tab[:, l, w, 2] = -np.tan(a_ry)
            tab[:, l, w, 4] = np.tan(a_rz)
            tab[:, l, w, 5] = -np.tan(a_rz)
            tab[:, l, w, 7] = np.tan(a_wy)
            tab[:, l, w, 8] = -np.tan(a_wy)
        tab[:, l, 0, 9] = lscale
    return tab.reshape(n, NCOLS).astype(np.float32)


def col(l, w, k):
    return (l * NQ + w) * NCPG + k


def postprocess(s64, action_scale, action_bias):
    """s64: (n, 64) block sums (blocks = top-6 bits). -> (n, NA) f32."""
    blk = np.arange(64)
    out = np.zeros((s64.shape[0], NA), np.float32)
    for w in range(NA):
        sign = 1.0 - 2.0 * ((blk >> (5 - w)) & 1)
        out[:, w] = s64 @ sign.astype(np.float32)
    return out * np.asarray(action_scale, np.float32) + np.asarray(
        action_bias, np.float32
    )


# ------------------------------------------------------------- gate schedule
# region = (buf, offset, dims) ; dims = tuple of (step, count), innermost last
# buf in {"r", "i", "T"}
# ops:
#   ("axpy", dst, y, cty, x): dst = col[cty]*y + x   (dst may alias x or y)
#   ("ts",   dst, src, c):    dst = col[c]*src       (DVE; dst==src ok)
#   ("tsr",  reg, c):         reg *= col[c]          (ScalarE, in place)
#   ("cp",   dst, src):       dst = src              (ScalarE; disjoint)
#   ("mul"/"add", dst, a, b): elementwise TT         (dst==a ok)
#   ("red",  dst, src): dst[.., k] = sum over innermost dim of src


def _rsize(reg):
    n = 1
    for _, c in reg[2]:
        n *= c
    return n


def _h(buf, w, bit):
    """half-region of wire w (bit==0/1), full state."""
    s = 1 << (13 - w)
    p = 1 << (14 - w)
    return (buf, bit * s, ((p, 1 << w), (1, s)))


def _full(buf):
    return (buf, 0, ((1, NS),))


def _conform(reg, like):
    """reshape a contiguous region to the dims-shape of `like` (same size)."""
    buf, off, dims = reg
    assert len(dims) == 1 and dims[0][0] == 1
    shape = tuple(c for _, c in like[2])
    ndims = []
    inner = 1
    for c in reversed(shape):
        ndims.append((inner, c))
        inner *= c
    return (buf, off, tuple(reversed(ndims)))


POOL_FRAC = float(os.environ.get("QK_POOL_FRAC", "0.33"))


class Sched:
    def __init__(self):
        self.ops = []
        self._debt = 0.0

    def _eng(self):
        """weighted round-robin: fraction POOL_FRAC of chains go to Pool."""
        self._debt += POOL_FRAC
        if self._debt >= 1.0:
            self._debt -= 1.0
            return 1
        return 0

    # shear pair: X' = X + (-t)*Y -> T (copy back), Y' = t*X + Y in place.
    # tslot: which half of T this chain owns (the two chains of a gate run
    # concurrently); sub-chunks of <=QT elems reuse the slot serially.
    def shear2(self, X, Y, ct, cnt, tslot=0):
        QT = HALF // 2
        Toff = tslot * QT
        eng = self._eng()
        for Xc, Yc in zip(_chunks(X, QT), _chunks(Y, QT), strict=True):
            n = _rsize(Xc)
            T = _conform(("T", Toff, ((1, n),)), Xc)
            self.ops.append(("axpy", T, Yc, cnt, Xc, eng))
            self.ops.append(("axpy", Yc, Xc, ct, Yc, eng))
            self.ops.append(("cp", Xc, T))

    def ry(self, w, ct, cnt):
        for s, P in enumerate(("r", "i")):
            self.shear2(_h(P, w, 0), _h(P, w, 1), ct, cnt, tslot=s)

    def rz(self, w, ct, cnt):
        # pairs (r,i): bit0 half rotates by +, bit1 half by - (conjugate)
        self.shear2(_h("r", w, 0), _h("i", w, 0), cnt, ct, tslot=0)
        self.shear2(_h("r", w, 1), _h("i", w, 1), ct, cnt, tslot=1)

    def scale_state(self, c):
        self.ops.append(("tsr", _full("r"), c))
        self.ops.append(("tsr", _full("i"), c))

    # layer-0 support-restricted variants (cosine scales deferred) -----------
    def ry_support(self, w, ct):
        s = 1 << (13 - w)
        p = 1 << (14 - w)
        for P in ("r", "i"):
            i0 = (P, 0, ((p, 1 << w),))
            i1 = (P, s, ((p, 1 << w),))
            self.ops.append(("ts", i1, i0, ct))

    def rz_support(self, w, ct, cnt):
        s = 1 << (13 - w)
        p = 1 << (14 - w)
        re_e = ("r", 0, ((p, 1 << w),))
        im_e = ("i", 0, ((p, 1 << w),))
        re_o = ("r", s, ((p, 1 << w),))
        im_o = ("i", s, ((p, 1 << w),))
        self.shear2(re_e, im_e, cnt, ct, tslot=0)
        self.shear2(re_o, im_o, ct, cnt, tslot=1)

    # RY_w(t) with CNOT(t-1, t) folded into writes ---------------------------
    # chain 0 = {bit_{t-1}=0 dests} on DVE, chain 1 = {bit_{t-1}=1} on Pool
    def ry_fold_cnot(self, t, ct, cnt):
        st = 1 << (13 - t)
        pt = 1 << (14 - t)
        P2 = pt * 2
        nb = 1 << (t - 1)

        def A(buf, a, b):
            return (buf, a * pt + b * st, ((P2, nb), (1, st)))

        n = nb * st  # 4096
        for P in ("r", "i"):
            e0, e1 = self._eng(), self._eng()
            T0 = _conform(("T", 0, ((1, n),)), A(P, 0, 0))
            T1 = _conform(("T", n, ((1, n),)), A(P, 0, 0))
            self.ops.append(("axpy", T0, A(P, 0, 1), cnt, A(P, 0, 0), e0))
            self.ops.append(("axpy", T1, A(P, 1, 1), cnt, A(P, 1, 0), e1))
            self.ops.append(("axpy", A(P, 0, 1), A(P, 0, 0), ct, A(P, 0, 1), e0))
            self.ops.append(("axpy", A(P, 1, 0), A(P, 1, 0), ct, A(P, 1, 1), e1))
            self.ops.append(("cp", A(P, 0, 0), T0))
            self.ops.append(("cp", A(P, 1, 1), T1))

    # RY_in(0) with previous layer's CNOT(13, 0) folded into reads -----------
    # chain 0 = bit13 even pairs on DVE, chain 1 = odd pairs on Pool
    def ry0_fold_cnot(self, ct, cnt):
        def A(buf, a, b):  # a = bit0 (MSB), b = bit13 (LSB)
            return (buf, a * HALF + b, ((2, HALF // 2),))

        Q = HALF // 2
        for P in ("r", "i"):
            e0, e1 = self._eng(), self._eng()
            Te = ("T", 0, ((1, Q),))
            To = ("T", Q, ((1, Q),))
            self.ops.append(("axpy", Te, A(P, 1, 0), cnt, A(P, 0, 0), e0))
            self.ops.append(("axpy", To, A(P, 0, 1), cnt, A(P, 1, 1), e1))
            self.ops.append(("axpy", A(P, 1, 0), A(P, 0, 0), ct, A(P, 1, 0), e0))
            self.ops.append(("axpy", A(P, 1, 1), A(P, 1, 1), ct, A(P, 0, 1), e1))
            self.ops.append(("cp", (P, 0, ((2, Q),)), Te))
            self.ops.append(("cp", (P, 1, ((2, Q),)), To))

    def cnot_13_0_explicit(self):
        for P in ("r", "i"):
            A01 = (P, 1, ((2, HALF // 2),))
            A11 = (P, HALF + 1, ((2, HALF // 2),))
            Tq = ("T", 0, ((1, HALF // 2),))
            self.ops.append(("cp", Tq, A01))
            self.ops.append(("cp", A01, A11))
            self.ops.append(("cp", A11, Tq))

    def measurement(self):
        self.ops.append(("sqsum", _full("r"), _full("r"), _full("i")))
        self.ops.append(("red", ("S", 0, ((1, 64),)), ("r", 0, ((256, 64), (1, 256)))))


def build_schedule():
    S = Sched()
    for l in range(NL):
        for w in range(NQ):
            kc = lambda k: col(l, w, k)  # noqa: E731
            if l == 0:
                S.ry_support(w, kc(1))
                S.rz_support(w, kc(4), kc(5))
            elif w == 0:
                S.ry0_fold_cnot(kc(1), kc(2))
                S.rz(w, kc(4), kc(5))
            else:
                S.ry(w, kc(1), kc(2))
                S.rz(w, kc(4), kc(5))
        # weight RY block with folded ring CNOTs
        S.ry(0, col(l, 0, 7), col(l, 0, 8))
        for t in range(1, NQ):
            S.ry_fold_cnot(t, col(l, t, 7), col(l, t, 8))
        S.scale_state(col(l, 0, 9))
    S.cnot_13_0_explicit()
    S.measurement()
    return S.ops


# ------------------------------------------------------------ numpy executor


def _indices(reg):
    _, off, dims = reg
    idx = np.array([0], np.int64)
    for st, ct in dims:
        idx = (idx[:, None] + (np.arange(ct, dtype=np.int64) * st)[None, :]).ravel()
    return off + idx


def simulate_numpy(tab):
    """tab: (n, NCOLS) f32 angle table -> (n, 64) block sums, fp32 ops."""
    n = tab.shape[0]
    bufs = {
        "r": np.zeros((n, NS), np.float32),
        "i": np.zeros((n, NS), np.float32),
        "T": np.zeros((n, HALF), np.float32),
        "S": np.zeros((n, 64), np.float32),
    }
    bufs["r"][:, 0] = 1.0
    A = tab
    for op in build_schedule():
        kind = op[0]
        if kind == "axpy":
            _, dst, y, cty, x = op[:5]
            v = (
                A[:, cty : cty + 1] * bufs[y[0]][:, _indices(y)]
                + bufs[x[0]][:, _indices(x)]
            ).astype(np.float32)
            bufs[dst[0]][:, _indices(dst)] = v
        elif kind == "ts":
            _, dst, src, c = op
            bufs[dst[0]][:, _indices(dst)] = (
                A[:, c : c + 1] * bufs[src[0]][:, _indices(src)]
            ).astype(np.float32)
        elif kind == "tsr":
            _, reg, c = op
            ix = _indices(reg)
            bufs[reg[0]][:, ix] = (A[:, c : c + 1] * bufs[reg[0]][:, ix]).astype(
                np.float32
            )
        elif kind == "cp":
            _, dst, src = op
            bufs[dst[0]][:, _indices(dst)] = bufs[src[0]][:, _indices(src)]
        elif kind == "mul":
            _, dst, a, b = op
            bufs[dst[0]][:, _indices(dst)] = (
                bufs[a[0]][:, _indices(a)] * bufs[b[0]][:, _indices(b)]
            ).astype(np.float32)
        elif kind == "add":
            _, dst, a, b = op
            bufs[dst[0]][:, _indices(dst)] = (
                bufs[a[0]][:, _indices(a)] + bufs[b[0]][:, _indices(b)]
            ).astype(np.float32)
        elif kind == "sqsum":
            _, dst, a, b = op
            bufs[dst[0]][:, _indices(dst)] = (
                bufs[a[0]][:, _indices(a)] ** 2 + bufs[b[0]][:, _indices(b)] ** 2
            ).astype(np.float32)
        elif kind == "red":
            _, dst, src = op
            v = bufs[src[0]][:, _indices(src)].reshape(n, 64, 256).sum(axis=2)
            bufs[dst[0]][:, _indices(dst)] = v.astype(np.float32)
        else:
            raise ValueError(kind)
    return bufs["S"].copy()


# ------------------------------------------------------------------ bass side

_EMIT_MODE = os.environ.get("QK_EMIT", "stt")  # "stt" | "custom" | "stock"
_CUSTOM_OPS = {}


def _register_op(name, spec):
    from concourse.dve_uop import DveOpSpec
    from concourse.dve_spec import lower
    from concourse import dve_ops
    from concourse.dve_ops import DveOp, OPS

    for op in OPS:
        if op.name == name:
            return op
    row = dve_ops._CUSTOM_DVE_ROW_BASE + len(OPS)
    shas = {}
    for ver in ("v3", "v4"):
        shas[ver] = DveOpSpec(
            name=name, opcode=row, uops=lower(spec, ver=ver), rd1_en=True
        ).sha(ver)
    op = DveOp(name, spec, subdim=False, uops_sha=shas)
    OPS.append(op)
    dve_ops._SUB_OPCODE_FOR_NAME[name] = row
    dve_ops.CUSTOM_DVE_SPECS[name] = spec
    return op


def _get_custom_ops():
    """Register fused DVE ops (idempotent): AXPY out = s0*in0 + in1,
    SQSUM out = in0^2 + in1^2."""
    if _CUSTOM_OPS:
        return _CUSTOM_OPS
    from concourse.dve_spec import Spec, Src0, Src1, C0, sq

    _CUSTOM_OPS["axpy"] = _register_op(
        "AXPY_ANT",
        Spec(
            body=Src0 * C0 + Src1,
            reference=lambda in0, in1, s0, s1, imm2: (
                np.asarray(in0, np.float32) * np.asarray(s0, np.float32)
                + np.asarray(in1, np.float32)
            ).astype(np.float32),
        ),
    )
    _CUSTOM_OPS["sqsum"] = _register_op(
        "SQSUM_ANT",
        Spec(
            body=sq(Src0) + sq(Src1),
            reference=lambda in0, in1, s0, s1, imm2: (
                np.asarray(in0, np.float32) ** 2 + np.asarray(in1, np.float32) ** 2
            ).astype(np.float32),
        ),
    )
    return _CUSTOM_OPS


def _ap(bass_mod, tiles, reg):
    """region -> bass AP on the tile's underlying tensor."""
    tile_ap = tiles[reg[0]]
    t = tile_ap.tensor
    part = list(tile_ap.ap)[0]
    dims = [[part[0], part[1]]] + [[s, c] for s, c in reg[2]]
    base = tile_ap.offset
    return bass_mod.AP(t, base + reg[1], dims)


def _chunks(reg, size=CHUNK):
    """split region into <=size-element pieces along blocks or inner run."""
    buf, off, dims = reg
    if len(dims) == 1:
        st, ct = dims[0]
        if ct <= size:
            return [reg]
        assert ct % size == 0
        return [(buf, off + k * size * st, ((st, size),)) for k in range(ct // size)]
    (bs, nb), (st, run) = dims
    assert st == 1
    if nb * run <= size:
        return [reg]
    if run >= size:
        assert run % size == 0
        out = []
        for b in range(nb):
            for k in range(run // size):
                out.append((buf, off + b * bs + k * size, ((1, size),)))
        return out
    bpc = max(1, size // run)
    out = []
    for b0 in range(0, nb, bpc):
        nbb = min(bpc, nb - b0)
        out.append((buf, off + b0 * bs, ((bs, nbb), (1, run))))
    return out


def build_bass():
    import concourse.bass as bass
    import concourse.mybir as mybir
    import concourse.tile as tile
    from concourse import bacc
    from contextlib import ExitStack

    f32 = mybir.dt.float32
    copy_fn = mybir.ActivationFunctionType.Copy
    nc = bacc.Bacc("TRN2", target_bir_lowering=False, debug=False)
    ang_d = nc.dram_tensor("ang", [BPC, NCOLS], f32, kind="ExternalInput").ap()
    out_d = nc.dram_tensor("out", [BPC, 64], f32, kind="ExternalOutput").ap()

    sched = build_schedule()
    use_custom = _EMIT_MODE == "custom"
    use_pool = os.environ.get("QK_POOL", "1") == "1"
    reps = int(os.environ.get("QK_REPS", "1"))
    cops = _get_custom_ops() if use_custom else None

    with tile.TileContext(nc) as tc, ExitStack() as ctx:
        state_p = ctx.enter_context(tc.tile_pool(name="state", bufs=1))
        tmp_p = ctx.enter_context(tc.tile_pool(name="tmp", bufs=1))
        io_p = ctx.enter_context(tc.tile_pool(name="io", bufs=2))

        re_t = state_p.tile([PT, NS], f32, tag="re")
        im_t = state_p.tile([PT, NS], f32, tag="im")
        T_t = tmp_p.tile([PT, HALF], f32, tag="T")
        T2_t = tmp_p.tile([PT, CHUNK], f32, tag="T2")
        for t in [tt for tt in range(NTILES) for _ in range(reps)]:
            ang_t = io_p.tile([PT, NCOLS], f32, tag="ang")
            s64_t = io_p.tile([PT, 64], f32, tag="s64")
            nc.sync.dma_start(ang_t[:], ang_d[t * PT : (t + 1) * PT, :])

            tiles = {"r": re_t[:], "i": im_t[:], "T": T_t[:], "S": s64_t[:]}
            nc.vector.memset(re_t[:, 0:1], 1.0)
            nc.vector.memset(im_t[:, 0:1], 0.0)

            def scal(c):
                return ang_t[:, c : c + 1]

            for op in sched:
                kind = op[0]
                if kind == "axpy":
                    _, dst, y, cty, x = op[:5]
                    eng = op[5] if len(op) > 5 else 0
                    if use_pool and eng == 1:
                        # Pool has no STT opcode: TS into T2, then TT add
                        n = _rsize(dst)
                        t2 = _conform(("T2", 0, ((1, n),)), dst)
                        t2ap = _ap(bass, {"T2": T2_t[:]}, t2)
                        nc.gpsimd.tensor_scalar_mul(
                            t2ap, _ap(bass, tiles, y), scal(cty)
                        )
                        nc.gpsimd.tensor_add(
                            _ap(bass, tiles, dst), _ap(bass, tiles, x), t2ap
                        )
                    elif _EMIT_MODE == "stt":
                        nc.vector.scalar_tensor_tensor(
                            _ap(bass, tiles, dst),
                            _ap(bass, tiles, y),
                            scal(cty),
                            _ap(bass, tiles, x),
                            mybir.AluOpType.mult,
                            mybir.AluOpType.add,
                        )
                    elif use_custom:
                        nc.vector._custom_dve(
                            cops["axpy"],
                            out=_ap(bass, tiles, dst),
                            in0=_ap(bass, tiles, y),
                            in1=_ap(bass, tiles, x),
                            s0=scal(cty),
                        )
                    else:
                        dcs = _chunks(dst)
                        ycs = _chunks(y)
                        xcs = _chunks(x)
                        for dch, ych, xch in zip(dcs, ycs, xcs, strict=True):
                            n = _rsize(dch)
                            t2 = _conform(("T2", 0, ((1, n),)), dch)
                            t2ap = _ap(bass, {"T2": T2_t[:]}, t2)
                            nc.vector.tensor_scalar_mul(
                                t2ap, _ap(bass, tiles, ych), scal(cty)
                            )
                            nc.vector.tensor_add(
                                _ap(bass, tiles, dch), _ap(bass, tiles, xch), t2ap
                            )
                elif kind == "ts":
                    _, dst, src, c = op
                    nc.vector.tensor_scalar_mul(
                        _ap(bass, tiles, dst), _ap(bass, tiles, src), scal(c)
                    )
                elif kind == "tsr":
                    _, reg, c = op
                    rap = _ap(bass, tiles, reg)
                    nc.scalar.activation(rap, rap, copy_fn, scale=scal(c))
                elif kind == "cp":
                    _, dst, src = op
                    nc.scalar.mul(_ap(bass, tiles, dst), _ap(bass, tiles, src), 1.0)
                elif kind == "mul":
                    _, dst, a, b = op
                    nc.vector.tensor_mul(
                        _ap(bass, tiles, dst), _ap(bass, tiles, a), _ap(bass, tiles, b)
                    )
                elif kind == "add":
                    _, dst, a, b = op
                    nc.vector.tensor_add(
                        _ap(bass, tiles, dst), _ap(bass, tiles, a), _ap(bass, tiles, b)
                    )
                elif kind == "sqsum":
                    _, dst, a, b = op
                    if use_custom:
                        nc.vector._custom_dve(
                            cops["sqsum"],
                            out=_ap(bass, tiles, dst),
                            in0=_ap(bass, tiles, a),
                            in1=_ap(bass, tiles, b),
                        )
                    else:
                        bap = _ap(bass, tiles, b)
                        beng = nc.gpsimd if use_pool else nc.vector
                        beng.tensor_mul(bap, bap, bap)
                        aap = _ap(bass, tiles, a)
                        nc.vector.tensor_mul(aap, aap, aap)
                        nc.vector.tensor_add(_ap(bass, tiles, dst), aap, bap)
                elif kind == "red":
                    _, dst, src = op
                    nc.vector.tensor_reduce(
                        _ap(bass, tiles, dst),
                        _ap(bass, tiles, src),
                        axis=mybir.AxisListType.X,
                        op=mybir.AluOpType.add,
                    )
            nc.sync.dma_start(out_d[t * PT : (t + 1) * PT, :], s64_t[:])
    nc.compile()
    return nc


_NC_CACHE = None


def run_cores(ang_full, trace=False, **kw):
    """ang_full: (B, NCOLS). Returns (B, 64) block sums + BassKernelResults."""
    global _NC_CACHE
    from concourse.bass_utils import run_bass_kernel_spmd

    if _NC_CACHE is None:
        _NC_CACHE = build_bass()
    nc = _NC_CACHE
    in_maps = [
        {"ang": np.ascontiguousarray(ang_full[c * BPC : (c + 1) * BPC])}
        for c in range(NCORES)
    ]
    last_err = None
    for attempt in range(3):
        try:
            res = run_bass_kernel_spmd(nc, in_maps, core_ids=list(range(NCORES)),
                                       trace=trace, **kw)
            break
        except Exception as e:  # device occasionally needs a cooldown
            last_err = e
            import time as _time

            _time.sleep(45 * (attempt + 1))
    else:
        raise last_err
    s64 = np.concatenate([r["out"] for r in res.results], axis=0)
    return s64, res


def kernel(x, input_scaling, weights, action_scale, action_bias):
    tab = angle_table(x, input_scaling, weights)
    s64, _ = run_cores(tab)
    return postprocess(s64, action_scale, action_bias)
